# revision 52
# baseline (speedup 1.0000x reference)
"""LocallyConnected2d Bass kernel for 8 Trainium2 NeuronCores.

Problem (hardcoded): x[16,32,64,64] f32, weight[64,64,32,32,3,3] f32,
bias[32,64,64] f32 -> out[16,32,64,64] f32.  stride=1, pad=1, dil=1.

Sharding: outH split across 8 cores (8 rows each).  Per core, per output
row h: 64 w-positions x 3 kernel-rows of matmuls [K=96,M=32]x[K,N=16]
accumulated in PSUM, K = (kernel-col j)*32 + inC c.  The kernel is HBM
-bandwidth bound on the per-position weights, so the design minimizes
bytes and keeps the (serialized) DMA-engine stream dense:
  - weights (the dominant stream) stored fp8 e3m4, scaled by 2^8 on host
    (output descaled by 2^-8 on host - exact).  Halves weight HBM bytes;
    x stays bf16 (all-fp8 would breach the accuracy budget).
  - x is DMAed once (unreplicated, [32c, hh, 66wp*16b] bf16) into
    partitions 0..31; the kernel-column-shifted copies for partition
    groups j=1,2 are made on-chip by DVE partition-shifted copies
    (4x perf mode), cutting x HBM bytes 3x.
  - bias is added during the PSUM->SBUF copy (DVE tensor_add with a
    stride-0 broadcast AP over batch), so no bias row rides the K dim.
  - weight DMAs are row-granular early and finer toward the end (halves
    for rows 4-6, thirds for row 7) so little dependent compute remains
    after the last weight byte lands, without letting the ~0.63us/DMA
    HWDGE descriptor-gen cost outrun the transfer stream.
  - out DMAs issue after all weight DMAs so weights win the DMA queue.

w-positions are processed in quads: position w = q*4+g is computed by a
matmul col-tiled to column group g (tile_position=(0,32g)), so the four
LDWEIGHTS+MATMUL streams of a quad run concurrently in the PE array.
PSUM tile is [128 = 4g x 32o, 16 quads x 16b] per output row.
"""

import numpy as np
import ml_dtypes

B, C, H, W = 16, 32, 64, 64
OC = 32
KH = KW = 3
NCORES = 8
RPC = H // NCORES  # rows per core = 8
NQ = 4  # quad size (PE col groups)
WSCALE = 2.0**8  # weight scale into fp8e3 range (max 15.08 < 15.5)
XSCALE = 2.0**1  # x scale (fp8e3 chunks; bf16 chunks carry it exactly)
WP = W + 2  # padded width positions per row

BF16 = ml_dtypes.bfloat16
F8E3 = ml_dtypes.float8_e3m4

# x tile chunking by padded row hh: chunk -> (hh0, hh1)
XCHUNKS = [(0, 3), (3, 7), (7, 10)]

_cache = {}


def _build_nc():
    import concourse.bass as bass
    import concourse.tile as tile
    from concourse import bacc, mybir

    nc = bacc.Bacc(
        "TRN2", target_bir_lowering=False, debug=False, num_devices=NCORES
    )
    f32 = mybir.dt.float32
    f16 = mybir.dt.float16
    bf16 = mybir.dt.bfloat16
    f8e3 = mybir.dt.float8e3

    # x, split by row-usage: the outer padded rows (hh 0-2 and 7-9,
    # chunks 0 and 2) feed only 50% of the contraction terms, so they
    # ride as fp8e3 (half bytes, modest extra quant error); the heavily
    # used middle rows (hh 3-6, chunk 1) stay bf16.  All values are
    # scaled by 2 (exact for bf16) so fp8/bf16 contributions match.
    # Partition c holds x[c, hh, wp, b] for the 64 non-pad columns
    # wp=1..64; the zero pad columns wp=0,65 are memset on-chip.
    xs8 = nc.dram_tensor("xs8", (32, 6, W * B), f8e3, kind="ExternalInput")
    xsb = nc.dram_tensor("xsb", (32, 4, W * B), bf16, kind="ExternalInput")
    # wt: [8, 96, 64*3*32] f8e3, scaled by 2^8; [h, j*32+c, (w*3+ik)*32+o].
    wt = nc.dram_tensor(
        "wt", (RPC, 96, W * KH * OC), f8e3, kind="ExternalInput"
    )
    # bsc: [128, 8*16] f16 = 2^9 * bias[o, h, w] at [g*32+o, h*16+q],
    # w = q*4+g; added during the PSUM->SBUF copy with a b-broadcast AP.
    bsc = nc.dram_tensor(
        "bsc", (4 * OC, RPC * (W // NQ)), f16, kind="ExternalInput"
    )
    # out: [8, 128, 16*16] f16 = 2^8 * out[h, g*32+o, q*16+b] with w = q*4+g
    out = nc.dram_tensor(
        "out", (RPC, 4 * OC, (W // NQ) * B), f16, kind="ExternalOutput"
    )

    with tile.TileContext(nc) as tc:
        with (
            tc.tile_pool(name="xpool", bufs=1) as xpool,
            tc.tile_pool(name="wpool", bufs=1) as wpool,
            tc.tile_pool(name="opool", bufs=1) as opool,
            tc.tile_pool(name="psum", bufs=3, space="PSUM") as ppool,
            tc.tile_pool(name="psum7", bufs=1, space="PSUM") as ppool7,
        ):
            # Per-chunk x tiles [96, rows, WP*16].  DMA x once into
            # partitions 0..31; DVE makes the j=1,2 column-shifted
            # copies into partitions 32..95.
            XDT = [f8e3, bf16, f8e3]  # chunk dtypes
            xtiles = []
            for ci, (h0, h1) in enumerate(XCHUNKS):
                r = h1 - h0
                t = xpool.tile([96, r, WP * B], XDT[ci], tag=f"x{ci}")
                xtiles.append(t)
            bt = xpool.tile([4 * OC, RPC * (W // NQ)], f16, tag="bias")
            nc.scalar.dma_start(bt[:], bsc[:, :])
            # x0 rides the gpsimd SWDGE ring: shorter first-transfer
            # latency than HWDGE and it keeps the HWDGE queue free for
            # the weight stream.
            XSRC = [  # chunk -> dram slice
                lambda: xs8[:, 0:3],
                lambda: xsb[:, :],
                lambda: xs8[:, 3:6],
            ]

            def load_x(ci):
                nc.sync.dma_start(
                    xtiles[ci][0:32, :, B : B + W * B], XSRC[ci]()
                )
                nc.vector.memset(xtiles[ci][0:32, :, 0:B], 0.0)
                nc.vector.memset(
                    xtiles[ci][0:32, :, B + W * B : WP * B], 0.0
                )

            load_x(0)

            def xcopies(ci):
                # fp8 chunks copy through a bf16 bitcast view (paired
                # elements) so DVE keeps its 4x 2-byte perf mode.
                t = xtiles[ci]
                f8 = XDT[ci] == f8e3
                for j in (1, 2):
                    dst = t[32 * j : 32 * (j + 1), :, 0 : W * B]
                    srcv = t[0:32, :, j * B : j * B + W * B]
                    if f8:
                        dst, srcv = dst.bitcast(bf16), srcv.bitcast(bf16)
                    nc.vector.tensor_copy(dst, srcv)

            def xslice(hh, w, k):
                for (h0, h1), t in zip(XCHUNKS, xtiles):
                    if h0 <= hh < h1:
                        return t[0:k, hh - h0, w * B : (w + 1) * B]
                raise AssertionError

            # Weight DMAs, one tile per quad-range piece: rows 0..3
            # whole, rows 4..6 in halves, row 7 in thirds.  Finer pieces
            # toward the end shorten "weight bytes not yet arrived while
            # their dependent compute remains" without letting the
            # ~0.63us/DMA HWDGE cost outrun the transfers.
            WPIECES = {h: [(0, 16)] for h in range(4)}
            WPIECES.update({h: [(0, 8), (8, 16)] for h in (4, 5, 6)})
            WPIECES[7] = [(0, 7), (7, 12), (12, 16)]
            wtiles = {h: [] for h in range(RPC)}  # [(q0, q1, tile), ...]

            def load_w(h):
                for pi, (q0, q1) in enumerate(WPIECES[h]):
                    c0, c1 = q0 * NQ * KH * OC, q1 * NQ * KH * OC
                    t = wpool.tile([96, c1 - c0], f8e3, tag=f"w{h}_{pi}")
                    nc.sync.dma_start(t[:], wt[h, :, c0:c1])
                    wtiles[h].append((q0 * NQ, q1 * NQ, t))

            load_x(1)
            for h in range(RPC):
                load_w(h)
                if h == 1:
                    load_x(2)

            def wslice(h, w, ik, k):
                for w0, w1, t in wtiles[h]:
                    if w0 <= w < w1:
                        return t[0:k, ((w - w0) * 3 + ik) * 32 :][:, 0:32]
                raise AssertionError

            # x replication copies for chunks 0,1 ahead of all PSUM
            # copies in the DVE queue; chunk 2 (needed from row 5) is
            # emitted after row 1 so rows 0-1's PSUM copies aren't stuck
            # behind it.
            xcopies(0)
            xcopies(1)

            def bias_bcast(h, q0, q1):
                # [128, q1-q0] bias slice with a stride-0 batch dim so it
                # broadcasts across the 16 batch columns of each quad.
                a = bt[0 : 4 * OC, h * (W // NQ) + q0 : h * (W // NQ) + q1]
                return bass.AP(a.tensor, a.offset, list(a.ap) + [[0, B]])

            def mm_quads(h, pt, q0, q1, pq0):
                for q in range(q0, q1):
                    for g in range(NQ):
                        w = q * NQ + g
                        for ik in range(KH):
                            nc.tensor.matmul(
                                pt[
                                    32 * g : 32 * (g + 1),
                                    (q - pq0) * B : (q - pq0 + 1) * B,
                                ],
                                wslice(h, w, ik, 96),
                                xslice(h + ik, w, 96),
                                start=(ik == 0),
                                stop=(ik == 2),
                                tile_position=(0, 32 * g),
                            )

            outs = []  # (dram row, sbuf tile) deferred out DMAs
            NQW = W // NQ  # 16 quads per row
            for h in range(RPC):
                ot = opool.tile([4 * OC, NQW * B], f16, tag=f"o{h}")
                if h == RPC - 1:
                    # separate PSUM tile + copy per weight piece (PSUM
                    # dependencies are tile-granular: sharing one tile
                    # would serialize piece k+1's matmuls on piece k's
                    # copy); one out DMA for the row.
                    for pi, (q0, q1) in enumerate(WPIECES[h]):
                        pt = ppool7.tile(
                            [4 * OC, (q1 - q0) * B], f32, tag=f"p7{pi}"
                        )
                        mm_quads(h, pt, q0, q1, q0)
                        nc.vector.tensor_add(
                            ot[:, q0 * B : q1 * B], pt[:],
                            bias_bcast(h, q0, q1),
                        )
                else:
                    pt = ppool.tile([4 * OC, NQW * B], f32)
                    mm_quads(h, pt, 0, NQW, 0)
                    nc.vector.tensor_add(ot[:], pt[:], bias_bcast(h, 0, NQW))
                outs.append((out[h], ot))
                if h == 1:
                    xcopies(2)

            # out DMAs issued after all weight DMAs so weight transfers
            # win the DMA-engine queue.
            for osl, ot in outs:
                nc.sync.dma_start(osl, ot[:])
    nc.compile()
    return nc


def _prep_inputs(x, weight, bias):
    """Host-side shard + layout prep.  Returns list of 8 per-core dicts."""
    # padded x, transposed to [c, hh, wp, b], scaled by 2 (f32 master)
    xp = np.zeros((C, H + 2, W + 2, B), dtype=np.float32)
    xp[:, 1 : H + 1, 1 : W + 1, :] = np.ascontiguousarray(
        x.transpose(1, 2, 3, 0) * np.float32(XSCALE)
    )

    # weight -> [h, j, c, w, ik, o], scaled into fp8e3 range
    wtr = np.ascontiguousarray(
        weight.transpose(0, 5, 3, 1, 4, 2) * np.float32(WSCALE)
    ).astype(F8E3)
    wtr = wtr.reshape(H, 96, W, KH, OC)
    btr = bias.astype(np.float32) * np.float32(WSCALE * XSCALE)  # [o,h,w]

    in_maps = []
    for i in range(NCORES):
        h0 = i * RPC
        xcore = xp[:, h0 : h0 + RPC + 2, :, :]
        wcore = wtr[h0 : h0 + RPC]
        # bsc[g*32+o, h*16+q] = 2^8 * bias[o, h0+h, q*4+g]
        bcore = btr[:, h0 : h0 + RPC]  # [o, h, w]
        bcore = bcore.reshape(OC, RPC, W // NQ, NQ)
        bcore = (
            np.ascontiguousarray(bcore.transpose(3, 0, 1, 2))  # [g, o, h, q]
            .reshape(4 * OC, RPC * (W // NQ))
            .astype(np.float16)
        )

        xn = xcore[:, :, 1 : W + 1]  # [32, 10, 64, 16] non-pad cols
        x8 = np.concatenate([xn[:, 0:3], xn[:, 7:10]], axis=1)
        in_maps.append(
            {
                "xs8": np.ascontiguousarray(
                    x8.astype(F8E3).reshape(32, 6, W * B)
                ),
                "xsb": np.ascontiguousarray(
                    xn[:, 3:7].astype(BF16).reshape(32, 4, W * B)
                ),
                "wt": np.ascontiguousarray(
                    wcore.reshape(RPC, 96, W * KH * OC)
                ),
                "bsc": bcore,
            }
        )
    return in_maps


def _run(in_maps, trace=False, tmpdir=None):
    from concourse.bass_utils import run_bass_kernel_spmd

    if "nc" not in _cache:
        _cache["nc"] = _build_nc()
    return run_bass_kernel_spmd(
        _cache["nc"], in_maps, list(range(NCORES)), trace=trace, tmpdir=tmpdir
    )


def _assemble(results):
    out = np.empty((B, OC, H, W), dtype=np.float32)
    inv = np.float32(1.0 / (WSCALE * XSCALE))
    for i in range(NCORES):
        # res: [h, g*32+o, q*16+b], w = q*4+g
        res = (
            results[i]["out"].astype(np.float32).reshape(RPC, NQ, OC, W // NQ, B)
            * inv
        )
        # -> out[b, o, h, q*4+g]
        out[:, :, i * RPC : (i + 1) * RPC, :] = res.transpose(
            4, 2, 0, 3, 1
        ).reshape(B, OC, RPC, W)
    return out


def kernel(x, weight, bias):
    x = np.asarray(x)
    weight = np.asarray(weight)
    bias = np.asarray(bias)
    in_maps = _prep_inputs(x, weight, bias)
    results = _run(in_maps).results
    return _assemble(results)


# revision 53
# speedup vs baseline: 1.0297x; 1.0297x over previous
"""LocallyConnected2d Bass kernel for 8 Trainium2 NeuronCores.

Problem (hardcoded): x[16,32,64,64] f32, weight[64,64,32,32,3,3] f32,
bias[32,64,64] f32 -> out[16,32,64,64] f32.  stride=1, pad=1, dil=1.

Sharding: outH split across 8 cores (8 rows each).  Per core, per output
row h: 64 w-positions x 3 kernel-rows of matmuls [K=96,M=32]x[K,N=16]
accumulated in PSUM, K = (kernel-col j)*32 + inC c.  The kernel is HBM
-bandwidth bound on the per-position weights, so the design minimizes
bytes and keeps the (serialized) DMA-engine stream dense:
  - weights (the dominant stream) stored fp8 e3m4, scaled by 2^8 on host
    (output descaled by 2^-8 on host - exact).  Halves weight HBM bytes;
    x stays bf16 (all-fp8 would breach the accuracy budget).
  - x is DMAed once (unreplicated, [32c, hh, 66wp*16b] bf16) into
    partitions 0..31; the kernel-column-shifted copies for partition
    groups j=1,2 are made on-chip by DVE partition-shifted copies
    (4x perf mode), cutting x HBM bytes 3x.
  - bias is added during the PSUM->SBUF copy (DVE tensor_add with a
    stride-0 broadcast AP over batch), so no bias row rides the K dim.
  - weight DMAs are row-granular early and finer toward the end (halves
    for rows 4-6, thirds for row 7) so little dependent compute remains
    after the last weight byte lands, without letting the ~0.63us/DMA
    HWDGE descriptor-gen cost outrun the transfer stream.
  - out DMAs issue after all weight DMAs so weights win the DMA queue.

w-positions are processed in quads: position w = q*4+g is computed by a
matmul col-tiled to column group g (tile_position=(0,32g)), so the four
LDWEIGHTS+MATMUL streams of a quad run concurrently in the PE array.
PSUM tile is [128 = 4g x 32o, 16 quads x 16b] per output row.
"""

import numpy as np
import ml_dtypes

B, C, H, W = 16, 32, 64, 64
OC = 32
KH = KW = 3
NCORES = 8
RPC = H // NCORES  # rows per core = 8
NQ = 4  # quad size (PE col groups)
WSCALE = 2.0**8  # weight scale into fp8e3 range (max 15.08 < 15.5)
XSCALE = 2.0**1  # x scale (fp8e3 chunks; bf16 chunks carry it exactly)
WP = W + 2  # padded width positions per row

BF16 = ml_dtypes.bfloat16
F8E3 = ml_dtypes.float8_e3m4

# x tile chunking by padded row hh: chunk -> (hh0, hh1)
XCHUNKS = [(0, 3), (3, 7), (7, 10)]

_cache = {}


def _build_nc():
    import concourse.bass as bass
    import concourse.tile as tile
    from concourse import bacc, mybir

    nc = bacc.Bacc(
        "TRN2", target_bir_lowering=False, debug=False, num_devices=NCORES
    )
    f32 = mybir.dt.float32
    f16 = mybir.dt.float16
    bf16 = mybir.dt.bfloat16
    f8e3 = mybir.dt.float8e3

    # x, split by row-usage: the outer padded rows (hh 0-2 and 7-9,
    # chunks 0 and 2) feed only 50% of the contraction terms, so they
    # ride as fp8e3 (half bytes, modest extra quant error); the heavily
    # used middle rows (hh 3-6, chunk 1) stay bf16.  All values are
    # scaled by 2 (exact for bf16) so fp8/bf16 contributions match.
    # Partition c holds x[c, hh, wp, b] for the 64 non-pad columns
    # wp=1..64; the zero pad columns wp=0,65 are memset on-chip.
    xs8 = nc.dram_tensor("xs8", (32, 6, W * B), f8e3, kind="ExternalInput")
    xsb = nc.dram_tensor("xsb", (32, 4, W * B), bf16, kind="ExternalInput")
    # wt: [8, 96, 64*3*32] f8e3, scaled by 2^8; [h, j*32+c, (w*3+ik)*32+o].
    wt = nc.dram_tensor(
        "wt", (RPC, 96, W * KH * OC), f8e3, kind="ExternalInput"
    )
    # bsc: [128, 8*16] f16 = 2^9 * bias[o, h, w] at [g*32+o, h*16+q],
    # w = q*4+g; added during the PSUM->SBUF copy with a b-broadcast AP.
    bsc = nc.dram_tensor(
        "bsc", (4 * OC, RPC * (W // NQ)), f16, kind="ExternalInput"
    )
    # out: [8, 128, 16*16] f16 = 2^8 * out[h, g*32+o, q*16+b] with w = q*4+g
    out = nc.dram_tensor(
        "out", (RPC, 4 * OC, (W // NQ) * B), f16, kind="ExternalOutput"
    )

    with tile.TileContext(nc) as tc:
        with (
            tc.tile_pool(name="xpool", bufs=1) as xpool,
            tc.tile_pool(name="wpool", bufs=1) as wpool,
            tc.tile_pool(name="opool", bufs=1) as opool,
            tc.tile_pool(name="psum", bufs=3, space="PSUM") as ppool,
            tc.tile_pool(name="psum7", bufs=1, space="PSUM") as ppool7,
        ):
            # Per-chunk x tiles [96, rows, WP*16].  DMA x once into
            # partitions 0..31; DVE makes the j=1,2 column-shifted
            # copies into partitions 32..95.
            XDT = [f8e3, bf16, f8e3]  # chunk dtypes
            xtiles = []
            for ci, (h0, h1) in enumerate(XCHUNKS):
                r = h1 - h0
                t = xpool.tile([96, r, WP * B], XDT[ci], tag=f"x{ci}")
                xtiles.append(t)
            bt = xpool.tile([4 * OC, RPC * (W // NQ)], f16, tag="bias")
            XSRC = [  # chunk -> dram slice
                lambda: xs8[:, 0:3],
                lambda: xsb[:, :],
                lambda: xs8[:, 3:6],
            ]

            def load_x(ci):
                nc.sync.dma_start(
                    xtiles[ci][0:32, :, B : B + W * B], XSRC[ci]()
                )
                nc.vector.memset(xtiles[ci][0:32, :, 0:B], 0.0)
                nc.vector.memset(
                    xtiles[ci][0:32, :, B + W * B : WP * B], 0.0
                )

            load_x(0)

            def xcopies(ci):
                # fp8 chunks copy through a bf16 bitcast view (paired
                # elements) so DVE keeps its 4x 2-byte perf mode.
                t = xtiles[ci]
                f8 = XDT[ci] == f8e3
                for j in (1, 2):
                    dst = t[32 * j : 32 * (j + 1), :, 0 : W * B]
                    srcv = t[0:32, :, j * B : j * B + W * B]
                    if f8:
                        dst, srcv = dst.bitcast(bf16), srcv.bitcast(bf16)
                    nc.vector.tensor_copy(dst, srcv)

            def xslice(hh, w, k):
                for (h0, h1), t in zip(XCHUNKS, xtiles):
                    if h0 <= hh < h1:
                        return t[0:k, hh - h0, w * B : (w + 1) * B]
                raise AssertionError

            # Weight DMAs, one tile per quad-range piece: rows 0..3
            # whole, rows 4..6 in halves, row 7 in thirds.  Finer pieces
            # toward the end shorten "weight bytes not yet arrived while
            # their dependent compute remains" without letting the
            # ~0.63us/DMA HWDGE cost outrun the transfers.
            WPIECES = {h: [(0, 16)] for h in range(4)}
            WPIECES.update({h: [(0, 8), (8, 16)] for h in (4, 5, 6)})
            WPIECES[7] = [(0, 7), (7, 12), (12, 16)]
            wtiles = {h: [] for h in range(RPC)}  # [(q0, q1, tile), ...]

            def load_w(h):
                for pi, (q0, q1) in enumerate(WPIECES[h]):
                    c0, c1 = q0 * NQ * KH * OC, q1 * NQ * KH * OC
                    t = wpool.tile([96, c1 - c0], f8e3, tag=f"w{h}_{pi}")
                    nc.sync.dma_start(t[:], wt[h, :, c0:c1])
                    wtiles[h].append((q0 * NQ, q1 * NQ, t))

            for h in range(RPC):
                load_w(h)
                if h == 0:
                    load_x(1)
                    load_x(2)
                if h == 1:
                    # bias on the sync ring here: late enough that its
                    # HWDGE hold doesn't delay w0/w1 descriptor-gen,
                    # early enough to beat the first row's bias-add.
                    nc.sync.dma_start(bt[:], bsc[:, :])

            def wslice(h, w, ik, k):
                for w0, w1, t in wtiles[h]:
                    if w0 <= w < w1:
                        return t[0:k, ((w - w0) * 3 + ik) * 32 :][:, 0:32]
                raise AssertionError

            # x replication copies for chunks 0,1 ahead of all PSUM
            # copies in the DVE queue; chunk 2 (needed from row 5) is
            # emitted after row 1 so rows 0-1's PSUM copies aren't stuck
            # behind it.
            xcopies(0)
            xcopies(1)

            def bias_bcast(h, q0, q1):
                # [128, q1-q0] bias slice with a stride-0 batch dim so it
                # broadcasts across the 16 batch columns of each quad.
                a = bt[0 : 4 * OC, h * (W // NQ) + q0 : h * (W // NQ) + q1]
                return bass.AP(a.tensor, a.offset, list(a.ap) + [[0, B]])

            def mm_quads(h, pt, q0, q1, pq0):
                for q in range(q0, q1):
                    for g in range(NQ):
                        w = q * NQ + g
                        for ik in range(KH):
                            nc.tensor.matmul(
                                pt[
                                    32 * g : 32 * (g + 1),
                                    (q - pq0) * B : (q - pq0 + 1) * B,
                                ],
                                wslice(h, w, ik, 96),
                                xslice(h + ik, w, 96),
                                start=(ik == 0),
                                stop=(ik == 2),
                                tile_position=(0, 32 * g),
                            )

            outs = []  # (dram row, sbuf tile) deferred out DMAs
            NQW = W // NQ  # 16 quads per row
            for h in range(RPC):
                ot = opool.tile([4 * OC, NQW * B], f16, tag=f"o{h}")
                if h == RPC - 1:
                    # separate PSUM tile + copy per weight piece (PSUM
                    # dependencies are tile-granular: sharing one tile
                    # would serialize piece k+1's matmuls on piece k's
                    # copy); one out DMA for the row.
                    for pi, (q0, q1) in enumerate(WPIECES[h]):
                        pt = ppool7.tile(
                            [4 * OC, (q1 - q0) * B], f32, tag=f"p7{pi}"
                        )
                        mm_quads(h, pt, q0, q1, q0)
                        nc.vector.tensor_add(
                            ot[:, q0 * B : q1 * B], pt[:],
                            bias_bcast(h, q0, q1),
                        )
                else:
                    pt = ppool.tile([4 * OC, NQW * B], f32)
                    mm_quads(h, pt, 0, NQW, 0)
                    nc.vector.tensor_add(ot[:], pt[:], bias_bcast(h, 0, NQW))
                outs.append((out[h], ot))
                if h == 1:
                    xcopies(2)

            # out DMAs issued after all weight DMAs so weight transfers
            # win the DMA-engine queue.
            for osl, ot in outs:
                nc.sync.dma_start(osl, ot[:])
    nc.compile()
    return nc


def _prep_inputs(x, weight, bias):
    """Host-side shard + layout prep.  Returns list of 8 per-core dicts."""
    # padded x, transposed to [c, hh, wp, b], scaled by 2 (f32 master)
    xp = np.zeros((C, H + 2, W + 2, B), dtype=np.float32)
    xp[:, 1 : H + 1, 1 : W + 1, :] = np.ascontiguousarray(
        x.transpose(1, 2, 3, 0) * np.float32(XSCALE)
    )

    # weight -> [h, j, c, w, ik, o], scaled into fp8e3 range
    wtr = np.ascontiguousarray(
        weight.transpose(0, 5, 3, 1, 4, 2) * np.float32(WSCALE)
    ).astype(F8E3)
    wtr = wtr.reshape(H, 96, W, KH, OC)
    btr = bias.astype(np.float32) * np.float32(WSCALE * XSCALE)  # [o,h,w]

    in_maps = []
    for i in range(NCORES):
        h0 = i * RPC
        xcore = xp[:, h0 : h0 + RPC + 2, :, :]
        wcore = wtr[h0 : h0 + RPC]
        # bsc[g*32+o, h*16+q] = 2^8 * bias[o, h0+h, q*4+g]
        bcore = btr[:, h0 : h0 + RPC]  # [o, h, w]
        bcore = bcore.reshape(OC, RPC, W // NQ, NQ)
        bcore = (
            np.ascontiguousarray(bcore.transpose(3, 0, 1, 2))  # [g, o, h, q]
            .reshape(4 * OC, RPC * (W // NQ))
            .astype(np.float16)
        )

        xn = xcore[:, :, 1 : W + 1]  # [32, 10, 64, 16] non-pad cols
        x8 = np.concatenate([xn[:, 0:3], xn[:, 7:10]], axis=1)
        in_maps.append(
            {
                "xs8": np.ascontiguousarray(
                    x8.astype(F8E3).reshape(32, 6, W * B)
                ),
                "xsb": np.ascontiguousarray(
                    xn[:, 3:7].astype(BF16).reshape(32, 4, W * B)
                ),
                "wt": np.ascontiguousarray(
                    wcore.reshape(RPC, 96, W * KH * OC)
                ),
                "bsc": bcore,
            }
        )
    return in_maps


def _run(in_maps, trace=False, tmpdir=None):
    from concourse.bass_utils import run_bass_kernel_spmd

    if "nc" not in _cache:
        _cache["nc"] = _build_nc()
    return run_bass_kernel_spmd(
        _cache["nc"], in_maps, list(range(NCORES)), trace=trace, tmpdir=tmpdir
    )


def _assemble(results):
    out = np.empty((B, OC, H, W), dtype=np.float32)
    inv = np.float32(1.0 / (WSCALE * XSCALE))
    for i in range(NCORES):
        # res: [h, g*32+o, q*16+b], w = q*4+g
        res = (
            results[i]["out"].astype(np.float32).reshape(RPC, NQ, OC, W // NQ, B)
            * inv
        )
        # -> out[b, o, h, q*4+g]
        out[:, :, i * RPC : (i + 1) * RPC, :] = res.transpose(
            4, 2, 0, 3, 1
        ).reshape(B, OC, RPC, W)
    return out


def kernel(x, weight, bias):
    x = np.asarray(x)
    weight = np.asarray(weight)
    bias = np.asarray(bias)
    in_maps = _prep_inputs(x, weight, bias)
    results = _run(in_maps).results
    return _assemble(results)


# revision 60
# speedup vs baseline: 1.0524x; 1.0220x over previous
"""LocallyConnected2d Bass kernel for 8 Trainium2 NeuronCores.

Problem (hardcoded): x[16,32,64,64] f32, weight[64,64,32,32,3,3] f32,
bias[32,64,64] f32 -> out[16,32,64,64] f32.  stride=1, pad=1, dil=1.

Sharding: outH split across 8 cores (8 rows each).  Per core, per output
row h: 64 w-positions x 3 kernel-rows of matmuls [K=96,M=32]x[K,N=16]
accumulated in PSUM, K = (kernel-col j)*32 + inC c.  The kernel is HBM
-bandwidth bound on the per-position weights, so the design minimizes
bytes and keeps the (serialized) DMA-engine stream dense:
  - weights (the dominant stream) stored fp8 e3m4, scaled by 2^8 on host
    (output descaled on host - exact).  Halves weight HBM bytes.
  - x is DMAed once (unreplicated) into partitions 0..31; the kernel-
    column-shifted copies for partition groups j=1,2 are made on-chip by
    DVE partition-shifted copies (4x perf mode), cutting x HBM bytes 3x.
    The outer padded rows (chunks 0,2; 50% of contraction terms) ride as
    fp8 e3m4, the heavily-used middle rows stay bf16 (all-fp8 x would
    leave too little margin under the accuracy gate).
  - dummy matmuls on a zeroed scratch tile warm the PE p-state ramp
    before the first real matmul arrives.
  - bias is added during the PSUM->SBUF copy (DVE scalar_tensor_tensor
    with a stride-0 broadcast AP over batch), so no bias row rides the
    K dim and the fp8 bias transfer is half-size.
  - weight DMAs are row-granular early and finer toward the end (halves
    for rows 4-6, thirds for row 7) so little dependent compute remains
    after the last weight byte lands, without letting the ~0.63us/DMA
    HWDGE descriptor-gen cost outrun the transfer stream.
  - out DMAs issue after all weight DMAs so weights win the DMA queue.

w-positions are processed in quads: position w = q*4+g is computed by a
matmul col-tiled to column group g (tile_position=(0,32g)), so the four
LDWEIGHTS+MATMUL streams of a quad run concurrently in the PE array.
PSUM tile is [128 = 4g x 32o, 16 quads x 16b] per output row.
"""

import numpy as np
import ml_dtypes

B, C, H, W = 16, 32, 64, 64
OC = 32
KH = KW = 3
NCORES = 8
RPC = H // NCORES  # rows per core = 8
NQ = 4  # quad size (PE col groups)
WSCALE = 2.0**8  # weight scale into fp8e3 range (max 15.08 < 15.5)
XSCALE = 2.0**1  # x scale (fp8e3 chunks; bf16 chunks carry it exactly)
WP = W + 2  # padded width positions per row

BF16 = ml_dtypes.bfloat16
F8E3 = ml_dtypes.float8_e3m4

# x tile chunking by padded row hh: chunk -> (hh0, hh1)
XCHUNKS = [(0, 3), (3, 7), (7, 10)]

_cache = {}


def _build_nc():
    import concourse.bass as bass
    import concourse.tile as tile
    from concourse import bacc, mybir

    nc = bacc.Bacc(
        "TRN2", target_bir_lowering=False, debug=False, num_devices=NCORES
    )
    f32 = mybir.dt.float32
    f16 = mybir.dt.float16
    bf16 = mybir.dt.bfloat16
    f8e3 = mybir.dt.float8e3

    # x, split by row-usage: the outer padded rows (hh 0-2 and 7-9,
    # chunks 0 and 2) feed only 50% of the contraction terms, so they
    # ride as fp8e3 (half bytes, modest extra quant error); the heavily
    # used middle rows (hh 3-6, chunk 1) stay bf16.  All values are
    # scaled by 2 (exact for bf16) so fp8/bf16 contributions match.
    # Partition c holds x[c, hh, wp, b] for the 64 non-pad columns
    # wp=1..64; the zero pad columns wp=0,65 are memset on-chip.
    xs8 = nc.dram_tensor("xs8", (32, 6, W * B), f8e3, kind="ExternalInput")
    xsb = nc.dram_tensor("xsb", (32, 4, W * B), bf16, kind="ExternalInput")
    # wt: [8, 96, 64*3*32] f8e3, scaled by 2^8; [h, j*32+c, (w*3+ik)*32+o].
    wt = nc.dram_tensor(
        "wt", (RPC, 96, W * KH * OC), f8e3, kind="ExternalInput"
    )
    # bsc: [128, 8*16] f8e3 = 2^8 * bias[o, h, w] at [g*32+o, h*16+q],
    # w = q*4+g.  The PSUM holds 2^9-scaled sums, so the copy applies
    # out = psum*0.5 + bias8 (scalar_tensor_tensor) and the host
    # descales by 2^-8 - all exact powers of two.
    bsc = nc.dram_tensor(
        "bsc", (4 * OC, RPC * (W // NQ)), f8e3, kind="ExternalInput"
    )
    # out: [8, 128, 16*16] f16 = 2^8 * out[h, g*32+o, q*16+b] with w = q*4+g
    out = nc.dram_tensor(
        "out", (RPC, 4 * OC, (W // NQ) * B), f16, kind="ExternalOutput"
    )

    with tile.TileContext(nc) as tc:
        with (
            tc.tile_pool(name="xpool", bufs=1) as xpool,
            tc.tile_pool(name="wpool", bufs=1) as wpool,
            tc.tile_pool(name="opool", bufs=1) as opool,
            tc.tile_pool(name="psum", bufs=3, space="PSUM") as ppool,
            tc.tile_pool(name="psum7", bufs=1, space="PSUM") as ppool7,
        ):
            # Per-chunk x tiles [96, rows, WP*16].  DMA x once into
            # partitions 0..31; DVE makes the j=1,2 column-shifted
            # copies into partitions 32..95.
            # PE p-state warmup: the cost model runs matmuls at half
            # speed until the PE has been busy ~3us.  Dummy matmuls on a
            # memset scratch tile bridge t~1us..first-real-matmul so the
            # real work runs at full speed.  NWARM tuned empirically.
            NWARM = 36
            wtile = xpool.tile([96, 160], bf16, tag="warm")
            nc.vector.memset(wtile[:], 0.0)
            pwarm = ppool7.tile([32, 128], f32, tag="pwarm")
            for _ in range(NWARM):
                nc.tensor.matmul(
                    pwarm[:], wtile[0:96, 0:32], wtile[0:96, 32:160],
                    start=True, stop=True,
                )

            XDT = [f8e3, bf16, f8e3]  # chunk dtypes
            xtiles = []
            for ci, (h0, h1) in enumerate(XCHUNKS):
                r = h1 - h0
                t = xpool.tile([96, r, WP * B], XDT[ci], tag=f"x{ci}")
                xtiles.append(t)
            bt = xpool.tile([4 * OC, RPC * (W // NQ)], f8e3, tag="bias")
            XSRC = [  # chunk -> dram slice
                lambda: xs8[:, 0:3],
                lambda: xsb[:, :],
                lambda: xs8[:, 3:6],
            ]

            def load_x(ci):
                nc.sync.dma_start(
                    xtiles[ci][0:32, :, B : B + W * B], XSRC[ci]()
                )
                nc.vector.memset(xtiles[ci][0:32, :, 0:B], 0.0)
                nc.vector.memset(
                    xtiles[ci][0:32, :, B + W * B : WP * B], 0.0
                )

            def xcopies(ci):
                # fp8 chunks copy through a bf16 bitcast view (paired
                # elements) so DVE keeps its 4x 2-byte perf mode.
                t = xtiles[ci]
                f8 = XDT[ci] == f8e3
                for j in (1, 2):
                    dst = t[32 * j : 32 * (j + 1), :, 0 : W * B]
                    srcv = t[0:32, :, j * B : j * B + W * B]
                    if f8:
                        dst, srcv = dst.bitcast(bf16), srcv.bitcast(bf16)
                    nc.vector.tensor_copy(dst, srcv)

            def xslice(hh, w, k):
                for (h0, h1), t in zip(XCHUNKS, xtiles):
                    if h0 <= hh < h1:
                        return t[0:k, hh - h0, w * B : (w + 1) * B]
                raise AssertionError

            # Weight DMAs, one tile per quad-range piece: rows 0..3
            # whole, rows 4..6 in halves, row 7 in thirds.  Finer pieces
            # toward the end shorten "weight bytes not yet arrived while
            # their dependent compute remains" without letting the
            # ~0.63us/DMA HWDGE cost outrun the transfers.
            WPIECES = {h: [(0, 16)] for h in range(4)}
            WPIECES.update({h: [(0, 8), (8, 16)] for h in (4, 5, 6)})
            WPIECES[7] = [(0, 7), (7, 12), (12, 16)]
            wtiles = {h: [] for h in range(RPC)}  # [(q0, q1, tile), ...]

            def load_w(h):
                for pi, (q0, q1) in enumerate(WPIECES[h]):
                    c0, c1 = q0 * NQ * KH * OC, q1 * NQ * KH * OC
                    t = wpool.tile([96, c1 - c0], f8e3, tag=f"w{h}_{pi}")
                    nc.sync.dma_start(t[:], wt[h, :, c0:c1])
                    wtiles[h].append((q0 * NQ, q1 * NQ, t))

            for h in range(RPC):
                load_w(h)
                if h == 0:
                    load_x(0)
                    load_x(1)
                    load_x(2)
                if h == 1:
                    # bias on the sync ring here: late enough that its
                    # HWDGE hold doesn't delay w0/w1 descriptor-gen,
                    # early enough to beat the first row's bias-add.
                    nc.sync.dma_start(bt[:], bsc[:, :])

            def wslice(h, w, ik, k):
                for w0, w1, t in wtiles[h]:
                    if w0 <= w < w1:
                        return t[0:k, ((w - w0) * 3 + ik) * 32 :][:, 0:32]
                raise AssertionError

            # x replication copies for chunks 0,1 ahead of all PSUM
            # copies in the DVE queue; chunk 2 (needed from row 5) is
            # emitted after row 1 so rows 0-1's PSUM copies aren't stuck
            # behind it.
            xcopies(0)
            xcopies(1)

            def bias_bcast(h, q0, q1):
                # [128, q1-q0] bias slice with a stride-0 batch dim so it
                # broadcasts across the 16 batch columns of each quad.
                a = bt[0 : 4 * OC, h * (W // NQ) + q0 : h * (W // NQ) + q1]
                return bass.AP(a.tensor, a.offset, list(a.ap) + [[0, B]])

            def mm_quads(h, pt, q0, q1, pq0):
                for q in range(q0, q1):
                    for g in range(NQ):
                        w = q * NQ + g
                        for ik in range(KH):
                            nc.tensor.matmul(
                                pt[
                                    32 * g : 32 * (g + 1),
                                    (q - pq0) * B : (q - pq0 + 1) * B,
                                ],
                                wslice(h, w, ik, 96),
                                xslice(h + ik, w, 96),
                                start=(ik == 0),
                                stop=(ik == 2),
                                tile_position=(0, 32 * g),
                            )

            outs = []  # (dram row, sbuf tile) deferred out DMAs
            NQW = W // NQ  # 16 quads per row
            for h in range(RPC):
                ot = opool.tile([4 * OC, NQW * B], f16, tag=f"o{h}")
                if h == RPC - 1:
                    # separate PSUM tile + copy per weight piece (PSUM
                    # dependencies are tile-granular: sharing one tile
                    # would serialize piece k+1's matmuls on piece k's
                    # copy); one out DMA for the row.
                    for pi, (q0, q1) in enumerate(WPIECES[h]):
                        pt = ppool7.tile(
                            [4 * OC, (q1 - q0) * B], f32, tag=f"p7{pi}"
                        )
                        mm_quads(h, pt, q0, q1, q0)
                        nc.vector.scalar_tensor_tensor(
                            ot[:, q0 * B : q1 * B], pt[:], 0.5,
                            bias_bcast(h, q0, q1),
                            mybir.AluOpType.mult, mybir.AluOpType.add,
                        )
                else:
                    pt = ppool.tile([4 * OC, NQW * B], f32)
                    mm_quads(h, pt, 0, NQW, 0)
                    nc.vector.scalar_tensor_tensor(
                        ot[:], pt[:], 0.5, bias_bcast(h, 0, NQW),
                        mybir.AluOpType.mult, mybir.AluOpType.add,
                    )
                outs.append((out[h], ot))
                if h == 1:
                    xcopies(2)

            # out DMAs issued after all weight DMAs so weight transfers
            # win the DMA-engine queue.
            for osl, ot in outs:
                nc.sync.dma_start(osl, ot[:])
    nc.compile()
    return nc


def _prep_inputs(x, weight, bias):
    """Host-side shard + layout prep.  Returns list of 8 per-core dicts."""
    # padded x, transposed to [c, hh, wp, b], scaled by 2 (f32 master)
    xp = np.zeros((C, H + 2, W + 2, B), dtype=np.float32)
    xp[:, 1 : H + 1, 1 : W + 1, :] = np.ascontiguousarray(
        x.transpose(1, 2, 3, 0) * np.float32(XSCALE)
    )

    # weight -> [h, j, c, w, ik, o], scaled into fp8e3 range
    wtr = np.ascontiguousarray(
        weight.transpose(0, 5, 3, 1, 4, 2) * np.float32(WSCALE)
    ).astype(F8E3)
    wtr = wtr.reshape(H, 96, W, KH, OC)
    btr = bias.astype(np.float32) * np.float32(WSCALE)  # [o,h,w]

    in_maps = []
    for i in range(NCORES):
        h0 = i * RPC
        xcore = xp[:, h0 : h0 + RPC + 2, :, :]
        wcore = wtr[h0 : h0 + RPC]
        # bsc[g*32+o, h*16+q] = 2^8 * bias[o, h0+h, q*4+g]
        bcore = btr[:, h0 : h0 + RPC]  # [o, h, w]
        bcore = bcore.reshape(OC, RPC, W // NQ, NQ)
        bcore = (
            np.ascontiguousarray(bcore.transpose(3, 0, 1, 2))  # [g, o, h, q]
            .reshape(4 * OC, RPC * (W // NQ))
            .astype(F8E3)
        )

        xn = xcore[:, :, 1 : W + 1]  # [32, 10, 64, 16] non-pad cols
        x8 = np.concatenate([xn[:, 0:3], xn[:, 7:10]], axis=1)
        in_maps.append(
            {
                "xs8": np.ascontiguousarray(
                    x8.astype(F8E3).reshape(32, 6, W * B)
                ),
                "xsb": np.ascontiguousarray(
                    xn[:, 3:7].astype(BF16).reshape(32, 4, W * B)
                ),
                "wt": np.ascontiguousarray(
                    wcore.reshape(RPC, 96, W * KH * OC)
                ),
                "bsc": bcore,
            }
        )
    return in_maps


def _run(in_maps, trace=False, tmpdir=None):
    from concourse.bass_utils import run_bass_kernel_spmd

    if "nc" not in _cache:
        _cache["nc"] = _build_nc()
    return run_bass_kernel_spmd(
        _cache["nc"], in_maps, list(range(NCORES)), trace=trace, tmpdir=tmpdir
    )


def _assemble(results):
    out = np.empty((B, OC, H, W), dtype=np.float32)
    inv = np.float32(1.0 / WSCALE)
    for i in range(NCORES):
        # res: [h, g*32+o, q*16+b], w = q*4+g
        res = (
            results[i]["out"].astype(np.float32).reshape(RPC, NQ, OC, W // NQ, B)
            * inv
        )
        # -> out[b, o, h, q*4+g]
        out[:, :, i * RPC : (i + 1) * RPC, :] = res.transpose(
            4, 2, 0, 3, 1
        ).reshape(B, OC, RPC, W)
    return out


def kernel(x, weight, bias):
    x = np.asarray(x)
    weight = np.asarray(weight)
    bias = np.asarray(bias)
    in_maps = _prep_inputs(x, weight, bias)
    results = _run(in_maps).results
    return _assemble(results)
